# revision 8
# baseline (speedup 1.0000x reference)
"""ESIM attention Bass kernel for Trainium2, 8-core data-parallel over batch.

Per batch b (L=512, D=768):
    S   = x1 @ x2^T                          [L, L]
    e1  = softmax(S, axis=1) ; xe1 = e1 @ x2
    e2  = softmax(S, axis=0) ; xe2 = e2 @ x1
Returns (xe1, xe2), each [32, 512, 768] float32.

Implementation (raw Block bass, explicit semaphores; per core 4 batches):
  DMA in -> DVE round f32->f32r -> PE transpose -> x1T,x2T [d,*] f32r
  PE: S = x1T.T @ x2T (f32r, ~13-bit mantissa, fp32 accumulate)
  DVE: m1 = -rowmax(S); ACT: U = exp(S - m1) -> bf16, Z1 = rowsum (fused)
  PE: ST = S^T (f32); DVE: m2; ACT: A2 = exp(ST - m2) -> bf16, Z2
  DVE: A2 *= 1/Z2 (exact transposed col-softmax)
  PE: UT = U^T (bf16); ACT copies out
  PE: xe2 = A2.T @ x1_bf16 ; xe1 = (UT.T @ x2_bf16) * (1/Z1)  (scale-after)
"""

import sys

if "/opt/trn_rl_repo" not in sys.path:
    sys.path.insert(0, "/opt/trn_rl_repo")

import numpy as np
from contextlib import ExitStack

P = 128
L = 512
D = 768
B_FULL = 32
N_CORES = 8
B_CORE = B_FULL // N_CORES  # 4
NI = L // P   # 4
ND = D // P   # 6

_compiled = None


class Stream:
    """Per-engine op list with python-side semaphore tick bookkeeping."""

    def __init__(self, name):
        self.name = name
        self.ops = []          # (emit_fn, waits[(sem_key, val)], inc(sem_key, amount) | None)
        self.tick = 0          # running count for this stream's own sem

    def add(self, emit, waits=(), inc=None):
        self.ops.append((emit, list(waits), inc))

    def add_inc(self, emit, waits=(), amount=1):
        """Add op that increments this stream's sem; returns new tick value."""
        self.tick += amount
        self.ops.append((emit, list(waits), (self.name, amount)))
        return self.tick


def _build():
    import concourse.bass as bass
    import concourse.mybir as mybir

    f32 = mybir.dt.float32
    f32r = mybir.dt.float32r
    bf16 = mybir.dt.bfloat16
    EXP = mybir.ActivationFunctionType.Exp
    MAX = mybir.AluOpType.max
    X = mybir.AxisListType.X

    nc = bass.Bass()
    x1 = nc.dram_tensor("x1", [B_CORE, L, D], f32, kind="ExternalInput")
    x2 = nc.dram_tensor("x2", [B_CORE, L, D], f32, kind="ExternalInput")
    o1 = nc.dram_tensor("o1", [B_CORE, L, D], f32, kind="ExternalOutput")
    o2 = nc.dram_tensor("o2", [B_CORE, L, D], f32, kind="ExternalOutput")
    xin = (x1, x2)

    ctx = ExitStack()

    def sbuf(name, shape, dt):
        return ctx.enter_context(nc.sbuf_tensor(name, shape, dt))

    def psum(name, shape, dt):
        return ctx.enter_context(nc.psum_tensor(name, shape, dt))

    identF = sbuf("identF", [P, P], f32)
    identR = sbuf("identR", [P, P], f32r)
    ident16 = sbuf("ident16", [P, P], bf16)
    # xf double-buffered by batch parity; [tensor][it]
    xf = [[[sbuf(f"xf{p}_{t}_{it}", [P, D], f32) for it in range(NI)]
           for t in range(2)] for p in range(2)]
    xr = [[sbuf(f"xr{t}_{it}", [P, D], f32r) for it in range(NI)] for t in range(2)]
    xbf = [[sbuf(f"xbf{t}_{it}", [P, D], bf16) for it in range(NI)] for t in range(2)]
    # xT double-buffered; index g: 0..5 = x1T d-tiles, 6..11 = x2T
    xT = [[sbuf(f"xT{p}_{g}", [P, L], f32r) for g in range(2 * ND)] for p in range(2)]
    S = [sbuf(f"S{it}", [P, L], f32) for it in range(NI)]
    ST = [sbuf(f"ST{jt}", [P, L], f32) for jt in range(NI)]
    U = [sbuf(f"U{it}", [P, L], bf16) for it in range(NI)]
    UT = [sbuf(f"UT{jt}", [P, L], bf16) for jt in range(NI)]
    A2 = [sbuf(f"A2{jt}", [P, L], bf16) for jt in range(NI)]
    xe1 = [sbuf(f"xe1_{it}", [P, D], f32) for it in range(NI)]
    xe2 = [sbuf(f"xe2_{it}", [P, D], f32) for it in range(NI)]
    m1 = [sbuf(f"m1_{it}", [P, 1], f32) for it in range(NI)]
    z1 = [sbuf(f"z1_{it}", [P, 1], f32) for it in range(NI)]
    rz1 = [sbuf(f"rz1_{it}", [P, 1], f32) for it in range(NI)]
    m2 = [sbuf(f"m2_{jt}", [P, 1], f32) for jt in range(NI)]
    z2 = [sbuf(f"z2_{jt}", [P, 1], f32) for jt in range(NI)]
    rz2 = [sbuf(f"rz2_{jt}", [P, 1], f32) for jt in range(NI)]

    pXP = [psum("pXPa", [P, L], f32r), psum("pXPb", [P, L], f32r)]
    pST = psum("pST", [P, L], f32)
    pUT = psum("pUT", [P, L], bf16)
    pMain = [psum("pMainA", [P, 512], f32), psum("pMainB", [P, 512], f32)]
    pTail = [psum("pTailA", [P, 256], f32)[:, :], psum("pTailB", [P, 256], f32)[:, :]]

    SY, GQ, DV, AC, PE = (Stream("sin"), Stream("gpsimd"), Stream("vector"),
                          Stream("scalar"), Stream("tensor"))
    SY_OUT = Stream("sout")  # counter only; ops live in SY

    # ---------------- schedule construction ----------------
    L_in = {}
    L_round = {}
    L_cast = {}
    L_xT = {}
    L_xpg = {}
    L_Scp = {}
    L_m1 = {}
    L_Smm = {}
    L_STx = {}
    L_STcp = {}
    L_m2 = {}
    L_Ue = {}
    L_A2e = {}
    L_A2n = {}
    L_UTx = {}
    L_UTcp = {}
    L_o1mm = {}
    L_o2mm = {}
    L_xe1cp = {}
    L_xe2cp = {}
    L_xpose_done = {}
    L_cast_done = {}
    L_round_done = {}
    L_stage2_done = {}
    bank_last_copy = {}   # psum region key -> (sem_key, tick) of last copy-out

    # identities: gpsimd builds f32; DVE casts
    t_ms = GQ.add_inc(lambda: nc.gpsimd.memset(identF[:], 0.0))
    GQ.add_inc(lambda: nc.gpsimd.affine_select(
        out=identF[:], in_=identF[:],
        compare_op=mybir.AluOpType.not_equal, fill=1.0, base=0,
        pattern=[[-1, P]], channel_multiplier=1),
        waits=[("gpsimd", t_ms)])
    t_idF = GQ.tick
    DV.add_inc(lambda: nc.vector.tensor_copy(identR[:], identF[:]),
               waits=[("gpsimd", t_idF)])
    DV.add_inc(lambda: nc.vector.tensor_copy(ident16[:], identF[:]))
    t_ident = DV.tick

    def in_dmas(b):
        p = b & 1
        for t in range(2):
            for it in range(NI):
                waits = []
                if b >= 2:
                    waits.append(("vector", L_round_done[b - 2]))
                src = xin[t]
                k = p * 2 * NI + t * NI + it
                def emit(t=t, it=it, b=b, p=p, src=src):
                    return nc.sync.dma_start(
                        xf[p][t][it][:], src[b, it * P:(it + 1) * P, :])
                SY.add(emit, waits=waits, inc=(f"sin{k}", 16))
                L_in[(b, t, it)] = (f"sin{k}", 16 * (b // 2 + 1))

    def out_dmas(b):
        for it in range(NI):
            def emit2(b=b, it=it):
                return nc.sync.dma_start(
                    o2[b, it * P:(it + 1) * P, :], xe2[it][:])
            SY.add(emit2, waits=[("vector", L_xe2cp[(b, it)])], inc=("sout", 16))
            SY_OUT.tick += 16

            def emit1(b=b, it=it):
                return nc.sync.dma_start(
                    o1[b, it * P:(it + 1) * P, :], xe1[it][:])
            SY.add(emit1, waits=[("vector", L_xe1cp[(b, it)])], inc=("sout", 16))
            SY_OUT.tick += 16

    def batch_compute(b):
        p = b & 1

        # --- DVE: roundings ---
        for t in range(2):
            for it in range(NI):
                waits = [L_in[(b, t, it)]]
                if b >= 1:
                    waits.append(("tensor", L_xpose_done[b - 1]))
                    waits.append(("gpsimd", L_cast_done[b - 1]))
                L_round[(b, t, it)] = DV.add_inc(
                    lambda t=t, it=it, p=p: nc.vector.tensor_copy(
                        xr[t][it][:], xf[p][t][it][:]),
                    waits=waits)
        L_round_done[b] = DV.tick

        # --- gpsimd: bf16 casts ---
        for t in range(2):
            for it in range(NI):
                waits = [("vector", L_round[(b, t, it)])]
                if b >= 1:
                    waits.append(("tensor", L_stage2_done[b - 1]))
                L_cast[(b, t, it)] = GQ.add_inc(
                    lambda t=t, it=it: nc.gpsimd.tensor_copy(
                        xbf[t][it][:], xr[t][it][:]),
                    waits=waits)
        L_cast_done[b] = GQ.tick

        # --- PE: x transposes (12 groups of 4 blocks) ---
        for g in range(2 * ND):
            t, dt = (0, g) if g < ND else (1, g - ND)
            bank = g & 1
            waits = [("vector", L_round[(b, t, NI - 1)])]
            key = ("xp", bank)
            if key in bank_last_copy:
                waits.append(bank_last_copy[key])
            if b == 0 and g < 2:
                waits.append(("vector", t_ident))
                waits.append(("gpsimd", t_idF))
            for it in range(NI):
                emit = (lambda t=t, dt=dt, it=it, bank=bank:
                        nc.tensor.transpose(
                            pXP[bank][:, it * P:(it + 1) * P],
                            xr[t][it][:, dt * P:(dt + 1) * P],
                            identR[:]))
                if it < NI - 1:
                    PE.add(emit, waits=waits if it == 0 else ())
                else:
                    L_xpg[(b, g)] = PE.add_inc(emit, waits=())
            # DVE copy out
            cwaits = [("tensor", L_xpg[(b, g)])]
            if b >= 2:
                pass  # xT WAR: S matmuls of b-2 long done (sp monotone via xpg waits)
            L_xT[(b, g)] = DV.add_inc(
                lambda g=g, bank=bank, p=p: nc.vector.tensor_copy(
                    xT[p][g][:], pXP[bank][:]),
                waits=cwaits)
            bank_last_copy[("xp", bank)] = ("vector", L_xT[(b, g)])
        L_xpose_done[b] = PE.tick

        # --- PE: S = x1 @ x2^T (uses stage-2 main banks) ; DVE: copy + m1 ---
        for it in range(NI):
            c = it & 1
            for dt in range(ND):
                waits = [("vector", L_xT[(b, ND + dt)])]
                if dt == 0:
                    key = ("main", c)
                    if key in bank_last_copy:
                        waits.append(bank_last_copy[key])
                emit = (lambda it=it, dt=dt, p=p, c=c: nc.tensor.matmul(
                    pMain[c][:],
                    xT[p][dt][:, it * P:(it + 1) * P],
                    xT[p][ND + dt][:],
                    start=(dt == 0), stop=(dt == ND - 1)))
                if dt < ND - 1:
                    PE.add(emit, waits=waits)
                else:
                    L_Smm[(b, it)] = PE.add_inc(emit, waits=waits)
            L_Scp[(b, it)] = DV.add_inc(
                lambda it=it, c=c: nc.vector.tensor_copy(S[it][:], pMain[c][:]),
                waits=[("tensor", L_Smm[(b, it)])])
            bank_last_copy[("main", c)] = ("vector", L_Scp[(b, it)])
            L_m1[(b, it)] = DV.add_inc(
                lambda it=it: nc.vector.tensor_reduce(
                    out=m1[it][:], in_=S[it][:], axis=X, op=MAX, negate=True),
                waits=[("vector", L_Scp[(b, it)])])

        # --- ACT: U = exp(S - m1) -> bf16, accum Z1 ---
        for it in range(NI):
            waits = [("vector", L_m1[(b, it)])]
            if b >= 1:
                waits.append(("tensor", L_UTx[(b - 1, NI - 1)]))
            L_Ue[(b, it)] = AC.add_inc(
                lambda it=it: nc.scalar.activation(
                    out=U[it][:], in_=S[it][:], func=EXP,
                    bias=m1[it][:], scale=1.0, accum_out=z1[it][:]),
                waits=waits)

        # --- PE: ST = S^T ; DVE: copy + m2 ---
        for jt in range(NI):
            waits = [("vector", L_Scp[(b, NI - 1)])]
            key = ("pST",)
            if key in bank_last_copy:
                waits.append(bank_last_copy[key])
            for it in range(NI):
                emit = (lambda jt=jt, it=it: nc.tensor.transpose(
                    pST[:, it * P:(it + 1) * P],
                    S[it][:, jt * P:(jt + 1) * P],
                    identF[:]))
                if it < NI - 1:
                    PE.add(emit, waits=waits if it == 0 else ())
                else:
                    L_STx[(b, jt)] = PE.add_inc(emit, waits=())
            L_STcp[(b, jt)] = DV.add_inc(
                lambda jt=jt: nc.vector.tensor_copy(ST[jt][:], pST[:]),
                waits=[("tensor", L_STx[(b, jt)])])
            bank_last_copy[("pST",)] = ("vector", L_STcp[(b, jt)])
            L_m2[(b, jt)] = DV.add_inc(
                lambda jt=jt: nc.vector.tensor_reduce(
                    out=m2[jt][:], in_=ST[jt][:], axis=X, op=MAX, negate=True),
                waits=[("vector", L_STcp[(b, jt)])])

        # --- ACT: A2 = exp(ST - m2) -> bf16, accum Z2; DVE: A2 *= 1/Z2 ---
        for jt in range(NI):
            L_A2e[(b, jt)] = AC.add_inc(
                lambda jt=jt: nc.scalar.activation(
                    out=A2[jt][:], in_=ST[jt][:], func=EXP,
                    bias=m2[jt][:], scale=1.0, accum_out=z2[jt][:]),
                waits=[("vector", L_m2[(b, jt)])])
            t_r2 = DV.add_inc(
                lambda jt=jt: nc.vector.reciprocal(out=rz2[jt][:], in_=z2[jt][:]),
                waits=[("scalar", L_A2e[(b, jt)])])
            L_A2n[(b, jt)] = DV.add_inc(
                lambda jt=jt: nc.vector.tensor_scalar_mul(
                    A2[jt][:], A2[jt][:], rz2[jt][:]),
                waits=[("vector", t_r2)])

        # --- PE: UT = U^T (bf16); ACT copies out ---
        for jt in range(NI):
            waits = [("scalar", L_Ue[(b, NI - 1)])]
            key = ("pUT",)
            if key in bank_last_copy:
                waits.append(bank_last_copy[key])
            for it in range(NI):
                emit = (lambda jt=jt, it=it: nc.tensor.transpose(
                    pUT[:, it * P:(it + 1) * P],
                    U[it][:, jt * P:(jt + 1) * P],
                    ident16[:]))
                if it < NI - 1:
                    PE.add(emit, waits=waits if it == 0 else ())
                else:
                    L_UTx[(b, jt)] = PE.add_inc(emit, waits=())
            L_UTcp[(b, jt)] = AC.add_inc(
                lambda jt=jt: nc.scalar.copy(UT[jt][:], pUT[:]),
                waits=[("tensor", L_UTx[(b, jt)])])
            bank_last_copy[("pUT",)] = ("scalar", L_UTcp[(b, jt)])

        # --- PE stage 2 + DVE copies ---
        chain = 0
        for it in range(NI):
            for which in (2, 1):   # xe2 first, then xe1
                c = chain & 1
                chain += 1
                lhs = A2 if which == 2 else UT
                rhs = xbf[0] if which == 2 else xbf[1]
                lsem, llab = (("vector", L_A2n), ("scalar", L_UTcp))[0 if which == 2 else 1]
                main, tail = pMain[c], pTail[c]
                waits0 = [(lsem, llab[(b, NI - 1)]),
                          ("gpsimd", L_cast[(b, 0 if which == 2 else 1, NI - 1)])]
                keym = ("main", c)
                if keym in bank_last_copy:
                    waits0.append(bank_last_copy[keym])
                for jt in range(NI):
                    PE.add(lambda it=it, jt=jt, lhs=lhs, rhs=rhs, main=main:
                           nc.tensor.matmul(
                               main[:],
                               lhs[jt][:, it * P:(it + 1) * P],
                               rhs[jt][:, 0:512],
                               start=(jt == 0), stop=(jt == NI - 1)),
                           waits=waits0 if jt == 0 else ())
                waitsT = []
                keyt = ("tail", c)
                if keyt in bank_last_copy:
                    waitsT.append(bank_last_copy[keyt])
                for jt in range(NI):
                    emit = (lambda it=it, jt=jt, lhs=lhs, rhs=rhs, tail=tail:
                            nc.tensor.matmul(
                                tail,
                                lhs[jt][:, it * P:(it + 1) * P],
                                rhs[jt][:, 512:D],
                                start=(jt == 0), stop=(jt == NI - 1)))
                    if jt < NI - 1:
                        PE.add(emit, waits=waitsT if jt == 0 else ())
                    else:
                        lab = PE.add_inc(emit, waits=())
                if which == 2:
                    L_o2mm[(b, it)] = lab
                else:
                    L_o1mm[(b, it)] = lab

                # DVE copy-out
                cwaits = [("tensor", lab)]
                if b >= 1:
                    cwaits.append(("sout", 16 * 8 * b))
                xe = xe2 if which == 2 else xe1
                if which == 1:
                    t_r1 = DV.add_inc(
                        lambda it=it: nc.vector.reciprocal(
                            out=rz1[it][:], in_=z1[it][:]), waits=cwaits)
                    DV.add_inc(
                        lambda it=it, main=main: nc.vector.tensor_scalar_mul(
                            xe1[it][:, 0:512], main[:], rz1[it][:]),
                        waits=[("vector", t_r1)])
                    lab2 = DV.add_inc(
                        lambda it=it, tail=tail: nc.vector.tensor_scalar_mul(
                            xe1[it][:, 512:D], tail, rz1[it][:]))
                    L_xe1cp[(b, it)] = lab2
                else:
                    DV.add_inc(
                        lambda it=it, main=main: nc.vector.tensor_copy(
                            xe2[it][:, 0:512], main[:]), waits=cwaits)
                    lab2 = DV.add_inc(
                        lambda it=it, tail=tail: nc.vector.tensor_copy(
                            xe2[it][:, 512:D], tail))
                    L_xe2cp[(b, it)] = lab2
                bank_last_copy[("main", c)] = ("vector", lab2)
                bank_last_copy[("tail", c)] = ("vector", lab2)
        L_stage2_done[b] = PE.tick

    # build global schedule: inputs prefetched one batch ahead
    in_dmas(0)
    in_dmas(1)
    for b in range(B_CORE):
        batch_compute(b)
        if b + 2 < B_CORE:
            in_dmas(b + 2)
        out_dmas(b)
    SY.add(None, waits=[("sout", 16 * 8 * B_CORE)])

    # ---------------- emission ----------------
    sem_ctx = ExitStack()
    with ctx, sem_ctx, nc.Block() as block:
        sems = {}
        for key in (["sout", "vector", "scalar", "tensor", "gpsimd"]
                    + [f"sin{k}" for k in range(4 * NI)]):
            sems[key] = sem_ctx.enter_context(nc.semaphore(f"sem_{key}"))

        def emit_stream(engine, stream):
            high = {}

            def run(eng):
                for emit, waits, inc in stream.ops:
                    for sem_key, val in waits:
                        if high.get(sem_key, 0) >= val:
                            continue
                        high[sem_key] = val
                        eng.wait_ge(sems[sem_key], val)
                    if emit is None:
                        continue
                    inst = emit()
                    if inc is not None:
                        sem_key, amount = inc
                        inst.then_inc(sems[sem_key], amount)
            return run

        block.sync(emit_stream("sync", SY))
        block.gpsimd(emit_stream("gpsimd", GQ))
        block.vector(emit_stream("vector", DV))
        block.scalar(emit_stream("scalar", AC))
        block.tensor(emit_stream("tensor", PE))

    return nc


def _get_compiled():
    global _compiled
    if _compiled is None:
        _compiled = _build()
    return _compiled


def kernel(x1: np.ndarray, x2: np.ndarray):
    from concourse.bass_utils import run_bass_kernel_spmd

    nc = _get_compiled()
    x1 = np.ascontiguousarray(x1, dtype=np.float32)
    x2 = np.ascontiguousarray(x2, dtype=np.float32)
    in_maps = []
    for c in range(N_CORES):
        sl = slice(c * B_CORE, (c + 1) * B_CORE)
        in_maps.append({"x1": x1[sl], "x2": x2[sl]})
    res = run_bass_kernel_spmd(nc, in_maps, list(range(N_CORES)))
    xe1 = np.concatenate([res.results[c]["o1"] for c in range(N_CORES)], axis=0)
    xe2 = np.concatenate([res.results[c]["o2"] for c in range(N_CORES)], axis=0)
    return xe1, xe2


# revision 10
# speedup vs baseline: 23484.3291x; 23484.3291x over previous
"""ESIM attention Bass kernel for Trainium2, 8-core data-parallel over batch.

Per batch b (L=512, D=768):
    S   = x1 @ x2^T                          [L, L]
    e1  = softmax(S, axis=1) ; xe1 = e1 @ x2
    e2  = softmax(S, axis=0) ; xe2 = e2 @ x1
Returns (xe1, xe2), each [32, 512, 768] float32.

Implementation (raw Block bass, explicit semaphores; per core 4 batches):
  DMA in -> DVE round f32->f32r -> PE transpose -> x1T,x2T [d,*] f32r
  PE: S = x1T.T @ x2T (f32r, ~13-bit mantissa, fp32 accumulate)
  DVE: m1 = -rowmax(S); ACT: U = exp(S - m1) -> bf16, Z1 = rowsum (fused)
  PE: ST = S^T (f32); DVE: m2; ACT: A2 = exp(ST - m2) -> bf16, Z2
  DVE: A2 *= 1/Z2 (exact transposed col-softmax)
  PE: UT = U^T (bf16); ACT copies out
  PE: xe2 = A2.T @ x1_bf16 ; xe1 = (UT.T @ x2_bf16) * (1/Z1)  (scale-after)
"""

import sys

if "/opt/trn_rl_repo" not in sys.path:
    sys.path.insert(0, "/opt/trn_rl_repo")

import numpy as np
from contextlib import ExitStack

P = 128
L = 512
D = 768
B_FULL = 32
N_CORES = 8
B_CORE = B_FULL // N_CORES  # 4
NI = L // P   # 4
ND = D // P   # 6

_compiled = None


class Stream:
    """Per-engine op list with python-side semaphore tick bookkeeping."""

    def __init__(self, name):
        self.name = name
        self.ops = []          # (emit_fn, waits[(sem_key, val)], inc(sem_key, amount) | None)
        self.tick = 0          # running count for this stream's own sem

    def add(self, emit, waits=(), inc=None):
        self.ops.append((emit, list(waits), inc))

    def add_inc(self, emit, waits=(), amount=1):
        """Add op that increments this stream's sem; returns new tick value."""
        self.tick += amount
        self.ops.append((emit, list(waits), (self.name, amount)))
        return self.tick


def _build():
    import concourse.bass as bass
    import concourse.mybir as mybir

    f32 = mybir.dt.float32
    f32r = mybir.dt.float32r
    bf16 = mybir.dt.bfloat16
    EXP = mybir.ActivationFunctionType.Exp
    MAX = mybir.AluOpType.max
    X = mybir.AxisListType.X

    nc = bass.Bass()
    x1 = nc.dram_tensor("x1", [B_CORE, L, D], f32, kind="ExternalInput")
    x2 = nc.dram_tensor("x2", [B_CORE, L, D], f32, kind="ExternalInput")
    o1 = nc.dram_tensor("o1", [B_CORE, L, D], f32, kind="ExternalOutput")
    o2 = nc.dram_tensor("o2", [B_CORE, L, D], f32, kind="ExternalOutput")
    xin = (x1, x2)

    ctx = ExitStack()

    def sbuf(name, shape, dt):
        return ctx.enter_context(nc.sbuf_tensor(name, shape, dt))

    def psum(name, shape, dt):
        return ctx.enter_context(nc.psum_tensor(name, shape, dt))

    identF = sbuf("identF", [P, P], f32)
    identR = sbuf("identR", [P, P], f32r)
    ident16 = sbuf("ident16", [P, P], bf16)
    # xf single-buffered; xr double-buffered by batch parity
    xf = [[sbuf(f"xf{t}_{it}", [P, D], f32) for it in range(NI)] for t in range(2)]
    xr = [[[sbuf(f"xr{p}_{t}_{it}", [P, D], f32r) for it in range(NI)]
           for t in range(2)] for p in range(2)]
    # xT double-buffered; index g: 0..5 = x1T d-tiles, 6..11 = x2T
    xT = [[sbuf(f"xT{p}_{g}", [P, L], f32r) for g in range(2 * ND)] for p in range(2)]
    S = [sbuf(f"S{it}", [P, L], f32) for it in range(NI)]
    ST = [sbuf(f"ST{jt}", [P, L], f32) for jt in range(NI)]
    U = [sbuf(f"U{it}", [P, L], f32r) for it in range(NI)]
    UT = [sbuf(f"UT{jt}", [P, L], f32r) for jt in range(NI)]
    A2 = [sbuf(f"A2{jt}", [P, L], f32r) for jt in range(NI)]
    xe1 = [sbuf(f"xe1_{it}", [P, D], f32) for it in range(NI)]
    xe2 = [sbuf(f"xe2_{it}", [P, D], f32) for it in range(NI)]
    m1 = [sbuf(f"m1_{it}", [P, 1], f32) for it in range(NI)]
    z1 = [sbuf(f"z1_{it}", [P, 1], f32) for it in range(NI)]
    rz1 = [sbuf(f"rz1_{it}", [P, 1], f32) for it in range(NI)]
    m2 = [sbuf(f"m2_{jt}", [P, 1], f32) for jt in range(NI)]
    z2 = [sbuf(f"z2_{jt}", [P, 1], f32) for jt in range(NI)]
    rz2 = [sbuf(f"rz2_{jt}", [P, 1], f32) for jt in range(NI)]

    pXP = [psum("pXPa", [P, L], f32r), psum("pXPb", [P, L], f32r)]
    pST = psum("pST", [P, L], f32)
    pUT = psum("pUT", [P, L], f32r)
    pMain = [psum("pMainA", [P, 512], f32), psum("pMainB", [P, 512], f32)]
    pTail = [psum("pTailA", [P, 256], f32)[:, :], psum("pTailB", [P, 256], f32)[:, :]]

    SY, GQ, DV, AC, PE = (Stream("sin"), Stream("gpsimd"), Stream("vector"),
                          Stream("scalar"), Stream("tensor"))
    SY_OUT = Stream("sout")  # counter only; ops live in SY

    # ---------------- schedule construction ----------------
    L_in = {}
    L_round = {}
    L_cast = {}
    L_xT = {}
    L_xpg = {}
    L_Scp = {}
    L_m1 = {}
    L_Smm = {}
    L_STx = {}
    L_STcp = {}
    L_m2 = {}
    L_Ue = {}
    L_A2e = {}
    L_A2n = {}
    L_UTx = {}
    L_UTcp = {}
    L_o1mm = {}
    L_o2mm = {}
    L_xe1cp = {}
    L_xe2cp = {}
    L_xpose_done = {}
    L_cast_done = {}
    L_round_done = {}
    L_stage2_done = {}
    bank_last_copy = {}   # psum region key -> (sem_key, tick) of last copy-out

    # identities: gpsimd builds f32; DVE casts
    t_ms = GQ.add_inc(lambda: nc.gpsimd.memset(identF[:], 0.0))
    GQ.add_inc(lambda: nc.gpsimd.affine_select(
        out=identF[:], in_=identF[:],
        compare_op=mybir.AluOpType.not_equal, fill=1.0, base=0,
        pattern=[[-1, P]], channel_multiplier=1),
        waits=[("gpsimd", t_ms)])
    t_idF = GQ.tick
    DV.add_inc(lambda: nc.vector.tensor_copy(identR[:], identF[:]),
               waits=[("gpsimd", t_idF)])
    DV.add_inc(lambda: nc.vector.tensor_copy(ident16[:], identF[:]))
    t_ident = DV.tick

    def in_dmas(b):
        for t in range(2):
            for it in range(NI):
                waits = []
                if b >= 1:
                    waits.append(("vector", L_round_done[b - 1]))
                src = xin[t]
                k = t * NI + it
                def emit(t=t, it=it, b=b, src=src):
                    return nc.sync.dma_start(
                        xf[t][it][:], src[b, it * P:(it + 1) * P, :])
                SY.add(emit, waits=waits, inc=(f"sin{k}", 16))
                L_in[(b, t, it)] = (f"sin{k}", 16 * (b + 1))

    def out_dmas(b):
        for it in range(NI):
            def emit2(b=b, it=it):
                return nc.sync.dma_start(
                    o2[b, it * P:(it + 1) * P, :], xe2[it][:])
            SY.add(emit2, waits=[("vector", L_xe2cp[(b, it)])], inc=("sout", 16))
            SY_OUT.tick += 16

            def emit1(b=b, it=it):
                return nc.sync.dma_start(
                    o1[b, it * P:(it + 1) * P, :], xe1[it][:])
            SY.add(emit1, waits=[("vector", L_xe1cp[(b, it)])], inc=("sout", 16))
            SY_OUT.tick += 16

    def batch_compute(b):
        p = b & 1

        # --- DVE: roundings (f32 -> f32r), xr double-buffered by parity ---
        for t in range(2):
            for it in range(NI):
                waits = [L_in[(b, t, it)]]
                if b >= 1:
                    waits.append(("tensor", L_xpose_done[b - 1]))
                L_round[(b, t, it)] = DV.add_inc(
                    lambda t=t, it=it, p=p: nc.vector.tensor_copy(
                        xr[p][t][it][:], xf[t][it][:]),
                    waits=waits)
        L_round_done[b] = DV.tick

        # --- PE: x transposes (12 groups of 4 blocks) ---
        for g in range(2 * ND):
            t, dt = (0, g) if g < ND else (1, g - ND)
            bank = g & 1
            waits = [("vector", L_round[(b, t, NI - 1)])]
            key = ("xp", bank)
            if key in bank_last_copy:
                waits.append(bank_last_copy[key])
            if b == 0 and g < 2:
                waits.append(("vector", t_ident))
                waits.append(("gpsimd", t_idF))
            for it in range(NI):
                emit = (lambda t=t, dt=dt, it=it, bank=bank, p=p:
                        nc.tensor.transpose(
                            pXP[bank][:, it * P:(it + 1) * P],
                            xr[p][t][it][:, dt * P:(dt + 1) * P],
                            identR[:]))
                if it < NI - 1:
                    PE.add(emit, waits=waits if it == 0 else ())
                else:
                    L_xpg[(b, g)] = PE.add_inc(emit, waits=())
            # DVE copy out
            cwaits = [("tensor", L_xpg[(b, g)])]
            if b >= 2:
                pass  # xT WAR: S matmuls of b-2 long done (sp monotone via xpg waits)
            L_xT[(b, g)] = DV.add_inc(
                lambda g=g, bank=bank, p=p: nc.vector.tensor_copy(
                    xT[p][g][:], pXP[bank][:]),
                waits=cwaits)
            bank_last_copy[("xp", bank)] = ("vector", L_xT[(b, g)])
        L_xpose_done[b] = PE.tick

        # --- PE: S = x1 @ x2^T (uses stage-2 main banks) ; DVE: copy + m1 ---
        for it in range(NI):
            c = it & 1
            for dt in range(ND):
                waits = [("vector", L_xT[(b, ND + dt)])]
                if dt == 0:
                    key = ("main", c)
                    if key in bank_last_copy:
                        waits.append(bank_last_copy[key])
                emit = (lambda it=it, dt=dt, p=p, c=c: nc.tensor.matmul(
                    pMain[c][:],
                    xT[p][dt][:, it * P:(it + 1) * P],
                    xT[p][ND + dt][:],
                    start=(dt == 0), stop=(dt == ND - 1)))
                if dt < ND - 1:
                    PE.add(emit, waits=waits)
                else:
                    L_Smm[(b, it)] = PE.add_inc(emit, waits=waits)
            L_Scp[(b, it)] = DV.add_inc(
                lambda it=it, c=c: nc.vector.tensor_copy(S[it][:], pMain[c][:]),
                waits=[("tensor", L_Smm[(b, it)])])
            bank_last_copy[("main", c)] = ("vector", L_Scp[(b, it)])
            L_m1[(b, it)] = DV.add_inc(
                lambda it=it: nc.vector.tensor_reduce(
                    out=m1[it][:], in_=S[it][:], axis=X, op=MAX, negate=True),
                waits=[("vector", L_Scp[(b, it)])])

        # --- ACT: U = exp(S - m1) -> bf16, accum Z1 ---
        for it in range(NI):
            waits = [("vector", L_m1[(b, it)])]
            if b >= 1:
                waits.append(("tensor", L_UTx[(b - 1, NI - 1)]))
            L_Ue[(b, it)] = AC.add_inc(
                lambda it=it: nc.scalar.activation(
                    out=U[it][:], in_=S[it][:], func=EXP,
                    bias=m1[it][:], scale=1.0, accum_out=z1[it][:]),
                waits=waits)

        # --- PE: ST = S^T ; DVE: copy + m2 ---
        for jt in range(NI):
            waits = [("vector", L_Scp[(b, NI - 1)])]
            key = ("pST",)
            if key in bank_last_copy:
                waits.append(bank_last_copy[key])
            for it in range(NI):
                emit = (lambda jt=jt, it=it: nc.tensor.transpose(
                    pST[:, it * P:(it + 1) * P],
                    S[it][:, jt * P:(jt + 1) * P],
                    identF[:]))
                if it < NI - 1:
                    PE.add(emit, waits=waits if it == 0 else ())
                else:
                    L_STx[(b, jt)] = PE.add_inc(emit, waits=())
            L_STcp[(b, jt)] = DV.add_inc(
                lambda jt=jt: nc.vector.tensor_copy(ST[jt][:], pST[:]),
                waits=[("tensor", L_STx[(b, jt)])])
            bank_last_copy[("pST",)] = ("vector", L_STcp[(b, jt)])
            L_m2[(b, jt)] = DV.add_inc(
                lambda jt=jt: nc.vector.tensor_reduce(
                    out=m2[jt][:], in_=ST[jt][:], axis=X, op=MAX, negate=True),
                waits=[("vector", L_STcp[(b, jt)])])

        # --- ACT: A2 = exp(ST - m2) -> bf16, accum Z2; DVE: A2 *= 1/Z2 ---
        for jt in range(NI):
            L_A2e[(b, jt)] = AC.add_inc(
                lambda jt=jt: nc.scalar.activation(
                    out=A2[jt][:], in_=ST[jt][:], func=EXP,
                    bias=m2[jt][:], scale=1.0, accum_out=z2[jt][:]),
                waits=[("vector", L_m2[(b, jt)])])
            t_r2 = DV.add_inc(
                lambda jt=jt: nc.vector.reciprocal(out=rz2[jt][:], in_=z2[jt][:]),
                waits=[("scalar", L_A2e[(b, jt)])])
            L_A2n[(b, jt)] = DV.add_inc(
                lambda jt=jt: nc.vector.tensor_scalar_mul(
                    A2[jt][:], A2[jt][:], rz2[jt][:]),
                waits=[("vector", t_r2)])

        # --- PE: UT = U^T (bf16); ACT copies out ---
        for jt in range(NI):
            waits = [("scalar", L_Ue[(b, NI - 1)])]
            key = ("pUT",)
            if key in bank_last_copy:
                waits.append(bank_last_copy[key])
            for it in range(NI):
                emit = (lambda jt=jt, it=it: nc.tensor.transpose(
                    pUT[:, it * P:(it + 1) * P],
                    U[it][:, jt * P:(jt + 1) * P],
                    identR[:]))
                if it < NI - 1:
                    PE.add(emit, waits=waits if it == 0 else ())
                else:
                    L_UTx[(b, jt)] = PE.add_inc(emit, waits=())
            L_UTcp[(b, jt)] = AC.add_inc(
                lambda jt=jt: nc.scalar.copy(UT[jt][:], pUT[:]),
                waits=[("tensor", L_UTx[(b, jt)])])
            bank_last_copy[("pUT",)] = ("scalar", L_UTcp[(b, jt)])

        # --- PE stage 2 + DVE copies ---
        chain = 0
        for it in range(NI):
            for which in (2, 1):   # xe2 first, then xe1
                c = chain & 1
                chain += 1
                lhs = A2 if which == 2 else UT
                rhs = xr[p][0] if which == 2 else xr[p][1]
                lsem, llab = (("vector", L_A2n), ("scalar", L_UTcp))[0 if which == 2 else 1]
                main, tail = pMain[c], pTail[c]
                waits0 = [(lsem, llab[(b, NI - 1)]),
                          ("vector", L_round[(b, 0 if which == 2 else 1, NI - 1)])]
                keym = ("main", c)
                if keym in bank_last_copy:
                    waits0.append(bank_last_copy[keym])
                for jt in range(NI):
                    PE.add(lambda it=it, jt=jt, lhs=lhs, rhs=rhs, main=main:
                           nc.tensor.matmul(
                               main[:],
                               lhs[jt][:, it * P:(it + 1) * P],
                               rhs[jt][:, 0:512],
                               start=(jt == 0), stop=(jt == NI - 1)),
                           waits=waits0 if jt == 0 else ())
                waitsT = []
                keyt = ("tail", c)
                if keyt in bank_last_copy:
                    waitsT.append(bank_last_copy[keyt])
                for jt in range(NI):
                    emit = (lambda it=it, jt=jt, lhs=lhs, rhs=rhs, tail=tail:
                            nc.tensor.matmul(
                                tail,
                                lhs[jt][:, it * P:(it + 1) * P],
                                rhs[jt][:, 512:D],
                                start=(jt == 0), stop=(jt == NI - 1)))
                    if jt < NI - 1:
                        PE.add(emit, waits=waitsT if jt == 0 else ())
                    else:
                        lab = PE.add_inc(emit, waits=())
                if which == 2:
                    L_o2mm[(b, it)] = lab
                else:
                    L_o1mm[(b, it)] = lab

                # DVE copy-out
                cwaits = [("tensor", lab)]
                if b >= 1:
                    cwaits.append(("sout", 16 * 8 * b))
                xe = xe2 if which == 2 else xe1
                if which == 1:
                    t_r1 = DV.add_inc(
                        lambda it=it: nc.vector.reciprocal(
                            out=rz1[it][:], in_=z1[it][:]), waits=cwaits)
                    DV.add_inc(
                        lambda it=it, main=main: nc.vector.tensor_scalar_mul(
                            xe1[it][:, 0:512], main[:], rz1[it][:]),
                        waits=[("vector", t_r1)])
                    lab2 = DV.add_inc(
                        lambda it=it, tail=tail: nc.vector.tensor_scalar_mul(
                            xe1[it][:, 512:D], tail, rz1[it][:]))
                    L_xe1cp[(b, it)] = lab2
                else:
                    DV.add_inc(
                        lambda it=it, main=main: nc.vector.tensor_copy(
                            xe2[it][:, 0:512], main[:]), waits=cwaits)
                    lab2 = DV.add_inc(
                        lambda it=it, tail=tail: nc.vector.tensor_copy(
                            xe2[it][:, 512:D], tail))
                    L_xe2cp[(b, it)] = lab2
                bank_last_copy[("main", c)] = ("vector", lab2)
                bank_last_copy[("tail", c)] = ("vector", lab2)
        L_stage2_done[b] = PE.tick

    # build global schedule: inputs prefetched one batch ahead
    in_dmas(0)
    for b in range(B_CORE):
        batch_compute(b)
        if b + 1 < B_CORE:
            in_dmas(b + 1)
        out_dmas(b)
    SY.add(None, waits=[("sout", 16 * 8 * B_CORE)])

    # ---------------- emission ----------------
    sem_ctx = ExitStack()
    with ctx, sem_ctx, nc.Block() as block:
        sems = {}
        for key in (["sout", "vector", "scalar", "tensor", "gpsimd"]
                    + [f"sin{k}" for k in range(2 * NI)]):
            sems[key] = sem_ctx.enter_context(nc.semaphore(f"sem_{key}"))

        def emit_stream(engine, stream):
            high = {}

            def run(eng):
                for emit, waits, inc in stream.ops:
                    for sem_key, val in waits:
                        if high.get(sem_key, 0) >= val:
                            continue
                        high[sem_key] = val
                        eng.wait_ge(sems[sem_key], val)
                    if emit is None:
                        continue
                    inst = emit()
                    if inc is not None:
                        sem_key, amount = inc
                        inst.then_inc(sems[sem_key], amount)
            return run

        block.sync(emit_stream("sync", SY))
        block.gpsimd(emit_stream("gpsimd", GQ))
        block.vector(emit_stream("vector", DV))
        block.scalar(emit_stream("scalar", AC))
        block.tensor(emit_stream("tensor", PE))

    return nc


def _get_compiled():
    global _compiled
    if _compiled is None:
        _compiled = _build()
    return _compiled


def kernel(x1: np.ndarray, x2: np.ndarray):
    from concourse.bass_utils import run_bass_kernel_spmd

    nc = _get_compiled()
    x1 = np.ascontiguousarray(x1, dtype=np.float32)
    x2 = np.ascontiguousarray(x2, dtype=np.float32)
    in_maps = []
    for c in range(N_CORES):
        sl = slice(c * B_CORE, (c + 1) * B_CORE)
        in_maps.append({"x1": x1[sl], "x2": x2[sl]})
    res = run_bass_kernel_spmd(nc, in_maps, list(range(N_CORES)))
    xe1 = np.concatenate([res.results[c]["o1"] for c in range(N_CORES)], axis=0)
    xe2 = np.concatenate([res.results[c]["o2"] for c in range(N_CORES)], axis=0)
    return xe1, xe2
